# revision 14
# baseline (speedup 1.0000x reference)
"""Trainium2 Bass kernel for nn_MixedLinear_KV (moe_routing, memory-bound).

Math: the reference computes
    x_mix = sum_m coef_a[m] * fake_quant(x, a_scales[m], AB[m])
    w_mix = sum_{i,j,n} coef_w[i,j,n] * fake_quant(pad_ij(W), w_scales[n], WB[n])
    b_mix = sum_{i,j} coef_b[i,j] * pad_ij(b)
    out   = x_mix @ w_mix.T + b_mix

With the benchmark inputs (a_scales == 1, x ~ N(0,1) so |x| < 7.5 always,
verified at runtime), both activation fake-quants reduce to rint(x), so
    out = rint(x) @ (s * w_mix).T + b_mix,   s = coef_a.sum()

w_mix/b_mix/s involve only the tiny [512,1024] weight and are computed on
host. The device does the heavy matmul, fed entirely in fp8:

  - q = rint(x) is a small integer (|q| <= 7 unpatched), so q * 2^-k is
    EXACT in fp8 e4m3 (pure exponent shift). The host streams q * 2^-k as
    fp8 -> 4 MiB/core instead of 16 MiB fp32.
  - W_eff = s * w_mix.T scaled by 2^k is split W' = hi + lo, both e4m3
    (lo = fp8(W' - fp8(W'))); end-to-end error measured 6.7e-4 vs the
    2e-2 gate. The 2^k factors cancel exactly in the product.
  - PE runs fp8 DoubleRow matmuls (256-deep contraction). Output is
    computed TRANSPOSED (psum [128 o, 512 t]) so the WEIGHT is the
    stationary operand: each LDWEIGHTS serves 8 matmuls and hides under
    the streams, and the per-o bias is a per-partition vector.
  - Eviction: one scalar-engine Identity op per tile (psum + bias -> f16);
    the vector engine is not used at all.
  - Host transposes the [512, 4096] f16 result back to [4096, 512] f32.

Sharding: data-parallel over the batch dim (8 batches -> 8 cores).
"""

import sys

sys.path.insert(0, "/opt/trn_rl_repo")

import json

import ml_dtypes
import numpy as np

import concourse.bass as bass
import concourse.mybir as mybir
from concourse import tile
from concourse.bass_utils import run_bass_kernel_spmd

# Problem constants (hardcoded per task contract)
B, S, D_IN, D_OUT = 8, 4096, 1024, 512
HS = [512, 768, 1024]
NH = [8, 12, 16]
NKV = 4
AB = [4, 8]
WB = [4, 8]
N_CORES = 8
KC = D_IN // 256  # 4 double-row chunks of 256
TB = 8  # token blocks of 512
OC = D_OUT // 128  # 4 output chunks of 128 partitions
LO_C = 2  # lo-pass chunks: top 512 residual-power rows (rel err 1.04e-2)
NCH = KC + LO_C  # 6 matmul chunks per psum tile
F8 = ml_dtypes.float8_e4m3


def _split_multi_waits(bir_bytes: bytes) -> bytes:
    """This container's walrus supports only one sem-wait per instruction;
    hoist extra waits onto preceding NoOps on the same engine."""
    bir = json.loads(bir_bytes)
    for fn in bir["functions"]:
        for bb in fn["blocks"]:
            new_insts = []
            for inst in bb["instructions"]:
                si = inst.get("sync_info") or {}
                ow = si.get("on_wait") or []
                if len(ow) > 1:
                    for k, w in enumerate(ow[:-1]):
                        new_insts.append(
                            {
                                "debug": inst.get("debug", 0),
                                "engine": inst["engine"],
                                "ins": [],
                                "outs": [],
                                "name": f"{inst['name']}_wsplit{k}",
                                "opcode": "NoOp",
                                "sync_info": {"on_wait": [w]},
                            }
                        )
                    si["on_wait"] = [ow[-1]]
                new_insts.append(inst)
            bb["instructions"] = new_insts
    return json.dumps(bir).encode()


def _host_fold_weights(weight, bias, mix_weights, a_scales, w_scales):
    """Mirror the reference's fp32 weight mixture exactly; return
    (w_effT_f32 [1024,512], b_mix_f32 [512], w_mix [512,1024])."""
    w32 = np.asarray(weight, np.float32)
    b32 = np.asarray(bias, np.float32)
    mw = np.asarray(mix_weights, np.float32).reshape(3, 3, 2, 2)
    w_sc = np.asarray(w_scales, np.float32)

    coef_a = mw.sum(axis=(0, 1, 3))  # [2]
    coef_w = mw.sum(axis=2)  # [3,3,2]
    coef_b = mw.sum(axis=(2, 3))  # [3,3]

    w_mix = np.zeros((D_OUT, D_IN), np.float32)
    b_mix = np.zeros((D_OUT,), np.float32)
    for i, h in enumerate(HS):
        for j, nh in enumerate(NH):
            out_dim = NKV * (h // nh)
            w_pad = np.zeros((D_OUT, D_IN), np.float32)
            w_pad[:out_dim, :h] = w32[:out_dim, :h]
            b_pad = np.zeros((D_OUT,), np.float32)
            b_pad[:out_dim] = b32[:out_dim]
            for n, wb in enumerate(WB):
                qn, qp = -(2 ** (wb - 1)), 2 ** (wb - 1) - 1
                xs = w_pad / w_sc[n]
                xc = np.clip(xs, np.float32(qn), np.float32(qp))
                fq = np.rint(xc) * w_sc[n]
                w_mix = w_mix + coef_w[i, j, n] * fq
            b_mix = b_mix + coef_b[i, j] * b_pad

    s = np.float64(coef_a[0]) + np.float64(coef_a[1])
    w_eff = s * w_mix.astype(np.float64)  # [512, 1024]
    w_effT = np.ascontiguousarray(w_eff.T).astype(np.float32)  # [1024, 512]
    return w_effT, b_mix, w_mix


def _build_nc():
    f32, f16, f8 = mybir.dt.float32, mybir.dt.float16, mybir.dt.float8e4
    DR = mybir.MatmulPerfMode.DoubleRow
    nc = bass.Bass("TRN2", target_bir_lowering=False, debug=False)

    # xq[p, kc, tbp, tb2, i, s]: fp8 of q*2^-k with PERMUTED row
    # k' = kc*256 + i*128 + p, token = (tbp*2 + tb2)*512 + s
    xq_d = nc.dram_tensor(
        "xq", [128, KC, TB // 2, 2, 2, 512], f8, kind="ExternalInput"
    ).ap()
    # wq[p, c, i, o]: chunks 0-3 = hi rows, 4-5 = lo rows 0..511 (top
    # residual-power rows after the permutation)
    wq_d = nc.dram_tensor("wq", [128, NCH, 2, D_OUT], f8, kind="ExternalInput").ap()
    # bv[p, oc]: bias for output row oc*128 + p
    bv_d = nc.dram_tensor("bv", [128, OC], f32, kind="ExternalInput").ap()
    # transposed output
    out_d = nc.dram_tensor("outT", [D_OUT, S], f16, kind="ExternalOutput").ap()

    with tile.TileContext(nc) as tc:
        with (
            tc.tile_pool(name="const", bufs=1) as cpool,
            tc.tile_pool(name="op", bufs=8) as opool,
            tc.tile_pool(name="ps", bufs=8, space="PSUM") as pspool,
        ):
            # wq chunk 0 heads the fast-starting sync queue (the first
            # matmul blocks on it); remaining weight chunks ride gpsimd;
            # x tiles stream on sync/scalar in consumption order
            wq_sb = {}
            xq_sb = {}
            for c in range(NCH):
                wq_sb[c] = cpool.tile([128, 2, D_OUT], f8, name=f"wq{c}")
            nc.scalar.dma_start(out=wq_sb[0][:], in_=wq_d[:, 0])
            for c in range(1, NCH):
                nc.gpsimd.dma_start(out=wq_sb[c][:], in_=wq_d[:, c])
            for kc in range(KC):
                for tbp in range(TB // 2):
                    t = cpool.tile([128, 2, 2, 512], f8, name=f"xq{kc}_{tbp}")
                    if kc == KC - 1:
                        # the last k-chunk is consumed latest; it can ride
                        # the late-starting gpsimd queue without stalling
                        dma_eng = nc.gpsimd
                    else:
                        dma_eng = nc.sync if tbp % 2 == 0 else nc.scalar
                    dma_eng.dma_start(out=t[:], in_=xq_d[:, kc, tbp])
                    xq_sb[(kc, 2 * tbp)] = t
                    xq_sb[(kc, 2 * tbp + 1)] = t
            bv_sb = cpool.tile([128, OC], f32)
            nc.gpsimd.dma_start(out=bv_sb[:], in_=bv_d[:])

            for oc in range(OC):
                ps = [pspool.tile([128, S // TB], f32, tag="ps", name=f"ps{oc}_{i}") for i in range(TB)]
                # oc 0 consumes chunks in DMA-arrival order (the gpsimd
                # queue delivers its x tiles last); accumulation order is
                # irrelevant to the result
                c_seq = list(range(NCH))
                for ci, c in enumerate(c_seq):
                    kc = c if c < KC else c - KC
                    last = ci == NCH - 1
                    lhsT = wq_sb[c][:, :, oc * 128 : (oc + 1) * 128]
                    for tb in range(TB):
                        nc.tensor.matmul(
                            ps[tb][:],
                            lhsT=lhsT,
                            rhs=xq_sb[(kc, tb)][:, tb % 2],
                            start=(ci == 0),
                            stop=last,
                            perf_mode=DR,
                        )
                        if last:
                            o_sb = opool.tile([128, 512], f16, tag="o")
                            # alternate evictions DVE/ACT, stores over
                            # two queues: halves the post-matmul tail
                            if tb % 2 == 0:
                                nc.vector.tensor_scalar_add(
                                    o_sb[:],
                                    ps[tb][:],
                                    bv_sb[:, oc : oc + 1],
                                )
                            else:
                                nc.scalar.activation(
                                    o_sb[:],
                                    ps[tb][:],
                                    mybir.ActivationFunctionType.Identity,
                                    bias=bv_sb[:, oc : oc + 1],
                                    scale=1.0,
                                )
                            if oc == OC - 1:
                                # last chunk: keep the slow-draining gpsimd
                                # queue out of the critical tail
                                dma_eng = nc.sync if tb % 2 == 0 else nc.scalar
                            else:
                                dma_eng = nc.gpsimd if tb % 2 == 0 else nc.sync
                            dma_eng.dma_start(
                                out=out_d[
                                    oc * 128 : (oc + 1) * 128,
                                    tb * 512 : (tb + 1) * 512,
                                ],
                                in_=o_sb[:],
                            )

    orig = nc.to_json_bytes
    nc.to_json_bytes = lambda: _split_multi_waits(orig())
    return nc


_NC_CACHE = None


def _fq32(x, scale, bits):
    """fp32 fake_quant forward value, matching the reference bitwise."""
    qn, qp = -(2 ** (bits - 1)), 2 ** (bits - 1) - 1
    xs = (np.asarray(x, np.float32) / np.float32(scale)).astype(np.float32)
    xc = np.clip(xs, np.float32(qn), np.float32(qp))
    return (np.rint(xc) * np.float32(scale)).astype(np.float32)


def _x_mix_ref(x, mix_weights, a_scales):
    """The reference's activation mixture, in fp32."""
    mw = np.asarray(mix_weights, np.float32).reshape(3, 3, 2, 2)
    coef_a = mw.sum(axis=(0, 1, 3))
    xm = coef_a[0] * _fq32(x, a_scales[0], AB[0])
    return (xm + coef_a[1] * _fq32(x, a_scales[1], AB[1])).astype(np.float32)


def _prepare_inputs(x, weight, bias, mix_weights, a_scales, w_scales):
    """Host-side prep shared by kernel() and test.py. Returns
    (in_maps, wsum32 [1024,512], w_mix, b_mix, inv_scale)."""
    w_effT, b_mix, w_mix = _host_fold_weights(
        weight, bias, mix_weights, a_scales, w_scales
    )

    maxw = max(float(np.abs(w_effT).max()), 1e-30)
    k = int(np.clip(np.floor(np.log2(16.0 / maxw)), 0, 9))
    sc = np.float32(2.0**k)
    inv = np.float32(2.0**-k)

    wS = w_effT * sc  # [1024, 512]
    # permute k-rows so the rows with the largest fp8 residual power come
    # first; the lo correction pass covers only the top 512 of them
    # (measured end-to-end rel err 1.04e-2 vs the 2e-2 gate)
    wlo_full = wS - wS.astype(F8).astype(np.float32)
    perm = np.argsort(-((wlo_full**2).sum(axis=1)), kind="stable")
    wS_p = np.ascontiguousarray(wS[perm])
    whi = wS_p.astype(F8)
    whi32 = whi.astype(np.float32)
    wlo = (wS_p - whi32).astype(F8)
    wlo32 = wlo.astype(np.float32)
    wsum_p = whi32.copy()
    wsum_p[: 256 * LO_C] += wlo32[: 256 * LO_C]
    wsum32 = np.empty_like(wsum_p)
    wsum32[perm] = wsum_p  # original-k indexing (for the host patch)

    def to_pio(w8):  # [256*n, 512] -> [128, n, 2, 512]
        n = w8.shape[0] // 256
        return w8.reshape(n, 2, 128, D_OUT).transpose(2, 0, 1, 3)

    wq = np.ascontiguousarray(
        np.concatenate([to_pio(whi), to_pio(wlo[: 256 * LO_C])], axis=1)
    )  # [128, NCH, 2i, 512]

    bv = np.ascontiguousarray(
        b_mix.reshape(OC, 128).T.astype(np.float32)
    )  # [128, OC]

    # q * 2^-k in fp8 with permuted k, laid out [p, kc, tbp, tb2, i, s]
    q8 = (np.rint(np.asarray(x, np.float32)) * inv).astype(F8)  # [B, S, D_IN]
    in_maps = []
    for b in range(N_CORES):
        xb = q8[b][:, perm].reshape(TB // 2, 2, 512, KC, 2, 128)
        xq = np.ascontiguousarray(xb.transpose(5, 3, 0, 1, 4, 2))
        in_maps.append({"xq": xq, "wq": wq, "bv": bv})
    return in_maps, wsum32, w_mix, b_mix, inv


def kernel(x, weight, bias, mix_weights, a_scales, w_scales):
    global _NC_CACHE
    x = np.asarray(x, np.float32)
    assert x.shape == (B, S, D_IN)
    a_sc = np.asarray(a_scales, np.float32)

    in_maps, wsum32, w_mix, b_mix, inv = _prepare_inputs(
        x, weight, bias, mix_weights, a_scales, w_scales
    )

    if not np.all(a_sc == np.float32(1.0)):
        # General-scale fallback (benchmark inputs always have a_scales == 1):
        # compute the reference mixture on host in fp32.
        x_mix = _x_mix_ref(x, mix_weights, a_scales)
        return (np.einsum("bsi,oi->bso", x_mix, w_mix) + b_mix).astype(np.float32)

    if _NC_CACHE is None:
        _NC_CACHE = _build_nc()
    nc = _NC_CACHE

    try:
        res = run_bass_kernel_spmd(nc, in_maps, list(range(N_CORES)))
    except Exception:
        # one retry for transient device errors
        res = run_bass_kernel_spmd(nc, in_maps, list(range(N_CORES)))
    out = np.stack(
        [
            np.asarray(res.results[b]["outT"]).astype(np.float32).T
            for b in range(N_CORES)
        ],
        axis=0,
    )

    # Exact host patch for |x| >= 7.49, where rint(x) differs from the
    # reference's clipped fake-quants (x ~ N(0,1) in the benchmark: never
    # triggers; keeps kernel() correct for arbitrary inputs).
    idx = np.argwhere(np.abs(x) >= 7.49)
    if len(idx):
        for b, t, i in idx:
            xv = x[b, t, i]
            ref_xmix = _x_mix_ref(xv, mix_weights, a_sc)
            # what the device computed for this element (same IEEE ops)
            dev_q = np.float32(F8.type(np.float32(np.rint(xv)) * inv))
            out[b, t, :] += ref_xmix * w_mix[:, i] - dev_q * wsum32[i, :]
    return out


# revision 16
# speedup vs baseline: 1.0450x; 1.0450x over previous
"""Trainium2 Bass kernel for nn_MixedLinear_KV (moe_routing, memory-bound).

Math: the reference computes
    x_mix = sum_m coef_a[m] * fake_quant(x, a_scales[m], AB[m])
    w_mix = sum_{i,j,n} coef_w[i,j,n] * fake_quant(pad_ij(W), w_scales[n], WB[n])
    b_mix = sum_{i,j} coef_b[i,j] * pad_ij(b)
    out   = x_mix @ w_mix.T + b_mix

With the benchmark inputs (a_scales == 1, x ~ N(0,1) so |x| < 7.5 always,
verified at runtime), both activation fake-quants reduce to rint(x), so
    out = rint(x) @ (s * w_mix).T + b_mix,   s = coef_a.sum()

w_mix/b_mix/s involve only the tiny [512,1024] weight and are computed on
host. The device does the heavy matmul, fed entirely in fp8:

  - q = rint(x) is a small integer (|q| <= 7 unpatched), so q * 2^-k is
    EXACT in fp8 e4m3 (pure exponent shift). The host streams q * 2^-k as
    fp8 -> 4 MiB/core instead of 16 MiB fp32.
  - W_eff = s * w_mix.T scaled by 2^k is quantized to e4m3 (hi), plus an
    e4m3 residual correction (lo) over only the 512 k-rows with the
    largest residual power (k-rows are permuted host-side so those rows
    form the first two 256-row chunks; contraction order is free). This
    cuts PE work to 1.5 passes; end-to-end rel err measured 1.04e-2
    against the 2e-2 gate. The 2^k factors cancel exactly in the product.
  - PE runs fp8 DoubleRow matmuls (256-deep contraction, 2x MAC rate;
    the only perf mode TRN2's ISA has - uint8 modes are NC_v2-only, all
    dtypes stream 1 moving column/cycle). Output is computed TRANSPOSED
    (psum [128 o, 512 t]) so the WEIGHT is the stationary operand: each
    LDWEIGHTS serves 8 matmuls and hides under the streams. Measured
    steady-state cadence: 216 ns per 512-column matmul (the hw floor).
  - Eviction: psum + per-partition bias -> f16, alternating between the
    vector engine (tensor_scalar add) and scalar engine (Identity
    activation) so the last output chunk drains in half the time; stores
    alternate DMA queues, keeping the slow-draining gpsimd queue out of
    the critical tail.
  - Host transposes the [512, 4096] f16 result back to [4096, 512] f32.

Sharding: data-parallel over the batch dim (8 batches -> 8 cores).
HW exec time: ~67 us (baseline 88.7 us); PE span 188 x 216 ns = 40.6 us.
"""

import sys

sys.path.insert(0, "/opt/trn_rl_repo")

import json

import ml_dtypes
import numpy as np

import concourse.bass as bass
import concourse.mybir as mybir
from concourse import tile
from concourse.bass_utils import run_bass_kernel_spmd

# Problem constants (hardcoded per task contract)
B, S, D_IN, D_OUT = 8, 4096, 1024, 512
HS = [512, 768, 1024]
NH = [8, 12, 16]
NKV = 4
AB = [4, 8]
WB = [4, 8]
N_CORES = 8
KC = D_IN // 256  # 4 double-row chunks of 256
TB = 8  # token blocks of 512
OC = D_OUT // 128  # 4 output chunks of 128 partitions
LO_C = 2  # lo-pass chunks: top 512 residual-power rows (rel err 1.04e-2)
NCH = KC + LO_C  # 6 matmul chunks per psum tile
F8 = ml_dtypes.float8_e4m3


def _split_multi_waits(bir_bytes: bytes) -> bytes:
    """This container's walrus supports only one sem-wait per instruction;
    hoist extra waits onto preceding NoOps on the same engine."""
    bir = json.loads(bir_bytes)
    for fn in bir["functions"]:
        for bb in fn["blocks"]:
            new_insts = []
            for inst in bb["instructions"]:
                si = inst.get("sync_info") or {}
                ow = si.get("on_wait") or []
                if len(ow) > 1:
                    for k, w in enumerate(ow[:-1]):
                        new_insts.append(
                            {
                                "debug": inst.get("debug", 0),
                                "engine": inst["engine"],
                                "ins": [],
                                "outs": [],
                                "name": f"{inst['name']}_wsplit{k}",
                                "opcode": "NoOp",
                                "sync_info": {"on_wait": [w]},
                            }
                        )
                    si["on_wait"] = [ow[-1]]
                new_insts.append(inst)
            bb["instructions"] = new_insts
    return json.dumps(bir).encode()


def _host_fold_weights(weight, bias, mix_weights, a_scales, w_scales):
    """Mirror the reference's fp32 weight mixture exactly; return
    (w_effT_f32 [1024,512], b_mix_f32 [512], w_mix [512,1024])."""
    w32 = np.asarray(weight, np.float32)
    b32 = np.asarray(bias, np.float32)
    mw = np.asarray(mix_weights, np.float32).reshape(3, 3, 2, 2)
    w_sc = np.asarray(w_scales, np.float32)

    coef_a = mw.sum(axis=(0, 1, 3))  # [2]
    coef_w = mw.sum(axis=2)  # [3,3,2]
    coef_b = mw.sum(axis=(2, 3))  # [3,3]

    w_mix = np.zeros((D_OUT, D_IN), np.float32)
    b_mix = np.zeros((D_OUT,), np.float32)
    for i, h in enumerate(HS):
        for j, nh in enumerate(NH):
            out_dim = NKV * (h // nh)
            w_pad = np.zeros((D_OUT, D_IN), np.float32)
            w_pad[:out_dim, :h] = w32[:out_dim, :h]
            b_pad = np.zeros((D_OUT,), np.float32)
            b_pad[:out_dim] = b32[:out_dim]
            for n, wb in enumerate(WB):
                qn, qp = -(2 ** (wb - 1)), 2 ** (wb - 1) - 1
                xs = w_pad / w_sc[n]
                xc = np.clip(xs, np.float32(qn), np.float32(qp))
                fq = np.rint(xc) * w_sc[n]
                w_mix = w_mix + coef_w[i, j, n] * fq
            b_mix = b_mix + coef_b[i, j] * b_pad

    s = np.float64(coef_a[0]) + np.float64(coef_a[1])
    w_eff = s * w_mix.astype(np.float64)  # [512, 1024]
    w_effT = np.ascontiguousarray(w_eff.T).astype(np.float32)  # [1024, 512]
    return w_effT, b_mix, w_mix


def _build_nc():
    f32, f16, f8 = mybir.dt.float32, mybir.dt.float16, mybir.dt.float8e4
    DR = mybir.MatmulPerfMode.DoubleRow
    nc = bass.Bass("TRN2", target_bir_lowering=False, debug=False)

    # xq[p, kc, tbp, tb2, i, s]: fp8 of q*2^-k with PERMUTED row
    # k' = kc*256 + i*128 + p, token = (tbp*2 + tb2)*512 + s
    xq_d = nc.dram_tensor(
        "xq", [128, KC, TB // 2, 2, 2, 512], f8, kind="ExternalInput"
    ).ap()
    # wq[p, c, i, o]: chunks 0-3 = hi rows, 4-5 = lo rows 0..511 (top
    # residual-power rows after the permutation)
    wq_d = nc.dram_tensor("wq", [128, NCH, 2, D_OUT], f8, kind="ExternalInput").ap()
    # bv[p, oc]: bias for output row oc*128 + p
    bv_d = nc.dram_tensor("bv", [128, OC], f32, kind="ExternalInput").ap()
    # transposed output
    out_d = nc.dram_tensor("outT", [D_OUT, S], f16, kind="ExternalOutput").ap()

    with tile.TileContext(nc) as tc:
        with (
            tc.tile_pool(name="const", bufs=1) as cpool,
            tc.tile_pool(name="op", bufs=8) as opool,
            tc.tile_pool(name="ps", bufs=8, space="PSUM") as pspool,
        ):
            # wq chunk 0 heads the fast-starting sync queue (the first
            # matmul blocks on it); remaining weight chunks ride gpsimd;
            # x tiles stream on sync/scalar in consumption order
            wq_sb = {}
            xq_sb = {}
            for c in range(NCH):
                wq_sb[c] = cpool.tile([128, 2, D_OUT], f8, name=f"wq{c}")
            nc.scalar.dma_start(out=wq_sb[0][:], in_=wq_d[:, 0])
            for c in range(1, NCH):
                nc.gpsimd.dma_start(out=wq_sb[c][:], in_=wq_d[:, c])
            for kc in range(KC):
                for tbp in range(TB // 2):
                    t = cpool.tile([128, 2, 2, 512], f8, name=f"xq{kc}_{tbp}")
                    dma_eng = nc.sync if tbp % 2 == 0 else nc.scalar
                    dma_eng.dma_start(out=t[:], in_=xq_d[:, kc, tbp])
                    xq_sb[(kc, 2 * tbp)] = t
                    xq_sb[(kc, 2 * tbp + 1)] = t
            bv_sb = cpool.tile([128, OC], f32)
            nc.gpsimd.dma_start(out=bv_sb[:], in_=bv_d[:])

            for oc in range(OC):
                ps = [pspool.tile([128, S // TB], f32, tag="ps", name=f"ps{oc}_{i}") for i in range(TB)]
                # oc 0 consumes chunks in DMA-arrival order (the gpsimd
                # queue delivers its x tiles last); accumulation order is
                # irrelevant to the result
                c_seq = list(range(NCH))
                for ci, c in enumerate(c_seq):
                    kc = c if c < KC else c - KC
                    last = ci == NCH - 1
                    lhsT = wq_sb[c][:, :, oc * 128 : (oc + 1) * 128]
                    for tb in range(TB):
                        nc.tensor.matmul(
                            ps[tb][:],
                            lhsT=lhsT,
                            rhs=xq_sb[(kc, tb)][:, tb % 2],
                            start=(ci == 0),
                            stop=last,
                            perf_mode=DR,
                        )
                        if last:
                            o_sb = opool.tile([128, 512], f16, tag="o")
                            # alternate evictions DVE/ACT, stores over
                            # two queues: halves the post-matmul tail
                            if tb % 2 == 0:
                                nc.vector.tensor_scalar_add(
                                    o_sb[:],
                                    ps[tb][:],
                                    bv_sb[:, oc : oc + 1],
                                )
                            else:
                                nc.scalar.activation(
                                    o_sb[:],
                                    ps[tb][:],
                                    mybir.ActivationFunctionType.Identity,
                                    bias=bv_sb[:, oc : oc + 1],
                                    scale=1.0,
                                )
                            if oc == OC - 1:
                                # last chunk: keep the slow-draining gpsimd
                                # queue out of the critical tail
                                dma_eng = nc.sync if tb % 2 == 0 else nc.scalar
                            else:
                                dma_eng = nc.gpsimd if tb % 2 == 0 else nc.sync
                            dma_eng.dma_start(
                                out=out_d[
                                    oc * 128 : (oc + 1) * 128,
                                    tb * 512 : (tb + 1) * 512,
                                ],
                                in_=o_sb[:],
                            )

    orig = nc.to_json_bytes
    nc.to_json_bytes = lambda: _split_multi_waits(orig())
    return nc


_NC_CACHE = None


def _fq32(x, scale, bits):
    """fp32 fake_quant forward value, matching the reference bitwise."""
    qn, qp = -(2 ** (bits - 1)), 2 ** (bits - 1) - 1
    xs = (np.asarray(x, np.float32) / np.float32(scale)).astype(np.float32)
    xc = np.clip(xs, np.float32(qn), np.float32(qp))
    return (np.rint(xc) * np.float32(scale)).astype(np.float32)


def _x_mix_ref(x, mix_weights, a_scales):
    """The reference's activation mixture, in fp32."""
    mw = np.asarray(mix_weights, np.float32).reshape(3, 3, 2, 2)
    coef_a = mw.sum(axis=(0, 1, 3))
    xm = coef_a[0] * _fq32(x, a_scales[0], AB[0])
    return (xm + coef_a[1] * _fq32(x, a_scales[1], AB[1])).astype(np.float32)


def _prepare_inputs(x, weight, bias, mix_weights, a_scales, w_scales):
    """Host-side prep shared by kernel() and test.py. Returns
    (in_maps, wsum32 [1024,512], w_mix, b_mix, inv_scale)."""
    w_effT, b_mix, w_mix = _host_fold_weights(
        weight, bias, mix_weights, a_scales, w_scales
    )

    maxw = max(float(np.abs(w_effT).max()), 1e-30)
    k = int(np.clip(np.floor(np.log2(16.0 / maxw)), 0, 9))
    sc = np.float32(2.0**k)
    inv = np.float32(2.0**-k)

    wS = w_effT * sc  # [1024, 512]
    # permute k-rows so the rows with the largest fp8 residual power come
    # first; the lo correction pass covers only the top 512 of them
    # (measured end-to-end rel err 1.04e-2 vs the 2e-2 gate)
    wlo_full = wS - wS.astype(F8).astype(np.float32)
    perm = np.argsort(-((wlo_full**2).sum(axis=1)), kind="stable")
    wS_p = np.ascontiguousarray(wS[perm])
    whi = wS_p.astype(F8)
    whi32 = whi.astype(np.float32)
    wlo = (wS_p - whi32).astype(F8)
    wlo32 = wlo.astype(np.float32)
    wsum_p = whi32.copy()
    wsum_p[: 256 * LO_C] += wlo32[: 256 * LO_C]
    wsum32 = np.empty_like(wsum_p)
    wsum32[perm] = wsum_p  # original-k indexing (for the host patch)

    def to_pio(w8):  # [256*n, 512] -> [128, n, 2, 512]
        n = w8.shape[0] // 256
        return w8.reshape(n, 2, 128, D_OUT).transpose(2, 0, 1, 3)

    wq = np.ascontiguousarray(
        np.concatenate([to_pio(whi), to_pio(wlo[: 256 * LO_C])], axis=1)
    )  # [128, NCH, 2i, 512]

    bv = np.ascontiguousarray(
        b_mix.reshape(OC, 128).T.astype(np.float32)
    )  # [128, OC]

    # q * 2^-k in fp8 with permuted k, laid out [p, kc, tbp, tb2, i, s]
    q8 = (np.rint(np.asarray(x, np.float32)) * inv).astype(F8)  # [B, S, D_IN]
    in_maps = []
    for b in range(N_CORES):
        xb = q8[b][:, perm].reshape(TB // 2, 2, 512, KC, 2, 128)
        xq = np.ascontiguousarray(xb.transpose(5, 3, 0, 1, 4, 2))
        in_maps.append({"xq": xq, "wq": wq, "bv": bv})
    return in_maps, wsum32, w_mix, b_mix, inv


def kernel(x, weight, bias, mix_weights, a_scales, w_scales):
    global _NC_CACHE
    x = np.asarray(x, np.float32)
    assert x.shape == (B, S, D_IN)
    a_sc = np.asarray(a_scales, np.float32)

    in_maps, wsum32, w_mix, b_mix, inv = _prepare_inputs(
        x, weight, bias, mix_weights, a_scales, w_scales
    )

    if not np.all(a_sc == np.float32(1.0)):
        # General-scale fallback (benchmark inputs always have a_scales == 1):
        # compute the reference mixture on host in fp32.
        x_mix = _x_mix_ref(x, mix_weights, a_scales)
        return (np.einsum("bsi,oi->bso", x_mix, w_mix) + b_mix).astype(np.float32)

    if _NC_CACHE is None:
        _NC_CACHE = _build_nc()
    nc = _NC_CACHE

    try:
        res = run_bass_kernel_spmd(nc, in_maps, list(range(N_CORES)))
    except Exception:
        # one retry for transient device errors
        res = run_bass_kernel_spmd(nc, in_maps, list(range(N_CORES)))
    out = np.stack(
        [
            np.asarray(res.results[b]["outT"]).astype(np.float32).T
            for b in range(N_CORES)
        ],
        axis=0,
    )

    # Exact host patch for |x| >= 7.49, where rint(x) differs from the
    # reference's clipped fake-quants (x ~ N(0,1) in the benchmark: never
    # triggers; keeps kernel() correct for arbitrary inputs).
    idx = np.argwhere(np.abs(x) >= 7.49)
    if len(idx):
        for b, t, i in idx:
            xv = x[b, t, i]
            ref_xmix = _x_mix_ref(xv, mix_weights, a_sc)
            # what the device computed for this element (same IEEE ops)
            dev_q = np.float32(F8.type(np.float32(np.rint(xv)) * inv))
            out[b, t, :] += ref_xmix * w_mix[:, i] - dev_q * wsum32[i, :]
    return out


# revision 17
# speedup vs baseline: 1.0558x; 1.0103x over previous
"""Trainium2 Bass kernel for nn_MixedLinear_KV (moe_routing, memory-bound).

Math: the reference computes
    x_mix = sum_m coef_a[m] * fake_quant(x, a_scales[m], AB[m])
    w_mix = sum_{i,j,n} coef_w[i,j,n] * fake_quant(pad_ij(W), w_scales[n], WB[n])
    b_mix = sum_{i,j} coef_b[i,j] * pad_ij(b)
    out   = x_mix @ w_mix.T + b_mix

With the benchmark inputs (a_scales == 1, x ~ N(0,1) so |x| < 7.5 always,
verified at runtime), both activation fake-quants reduce to rint(x), so
    out = rint(x) @ (s * w_mix).T + b_mix,   s = coef_a.sum()

w_mix/b_mix/s involve only the tiny [512,1024] weight and are computed on
host. The device does the heavy matmul, fed entirely in fp8:

  - q = rint(x) is a small integer (|q| <= 7 unpatched), so q * 2^-k is
    EXACT in fp8 e4m3 (pure exponent shift). The host streams q * 2^-k as
    fp8 -> 4 MiB/core instead of 16 MiB fp32.
  - W_eff = s * w_mix.T scaled by 2^k is quantized to e4m3 (hi), plus an
    e4m3 residual correction (lo) over only the 512 k-rows with the
    largest residual power (k-rows are permuted host-side so those rows
    form the first two 256-row chunks; contraction order is free). This
    cuts PE work to 1.5 passes; end-to-end rel err measured 1.04e-2
    against the 2e-2 gate. The 2^k factors cancel exactly in the product.
  - PE runs fp8 DoubleRow matmuls (256-deep contraction, 2x MAC rate;
    the only perf mode TRN2's ISA has - uint8 modes are NC_v2-only, all
    dtypes stream 1 moving column/cycle). Output is computed TRANSPOSED
    (psum [128 o, 512 t]) so the WEIGHT is the stationary operand: each
    LDWEIGHTS serves 8 matmuls and hides under the streams. Measured
    steady-state cadence: 216 ns per 512-column matmul (the hw floor).
  - Eviction: psum + per-partition bias -> f16, alternating between the
    vector engine (tensor_scalar add) and scalar engine (Identity
    activation) so the last output chunk drains in half the time; stores
    alternate DMA queues, keeping the slow-draining gpsimd queue out of
    the critical tail.
  - Host transposes the [512, 4096] f16 result back to [4096, 512] f32.

Sharding: data-parallel over the batch dim (8 batches -> 8 cores).
HW exec time: ~67 us (baseline 88.7 us); PE span 188 x 216 ns = 40.6 us.
"""

import sys

sys.path.insert(0, "/opt/trn_rl_repo")

import json

import ml_dtypes
import numpy as np

import concourse.bass as bass
import concourse.mybir as mybir
from concourse import tile
from concourse.bass_utils import run_bass_kernel_spmd

# Problem constants (hardcoded per task contract)
B, S, D_IN, D_OUT = 8, 4096, 1024, 512
HS = [512, 768, 1024]
NH = [8, 12, 16]
NKV = 4
AB = [4, 8]
WB = [4, 8]
N_CORES = 8
KC = D_IN // 256  # 4 double-row chunks of 256
TB = 8  # token blocks of 512
OC = D_OUT // 128  # 4 output chunks of 128 partitions
LO_C = 2  # lo-pass chunks: top 512 residual-power rows (rel err 1.04e-2)
NCH = KC + LO_C  # 6 matmul chunks per psum tile
F8 = ml_dtypes.float8_e4m3


def _split_multi_waits(bir_bytes: bytes) -> bytes:
    """This container's walrus supports only one sem-wait per instruction;
    hoist extra waits onto preceding NoOps on the same engine."""
    bir = json.loads(bir_bytes)
    for fn in bir["functions"]:
        for bb in fn["blocks"]:
            new_insts = []
            for inst in bb["instructions"]:
                si = inst.get("sync_info") or {}
                ow = si.get("on_wait") or []
                if len(ow) > 1:
                    for k, w in enumerate(ow[:-1]):
                        new_insts.append(
                            {
                                "debug": inst.get("debug", 0),
                                "engine": inst["engine"],
                                "ins": [],
                                "outs": [],
                                "name": f"{inst['name']}_wsplit{k}",
                                "opcode": "NoOp",
                                "sync_info": {"on_wait": [w]},
                            }
                        )
                    si["on_wait"] = [ow[-1]]
                new_insts.append(inst)
            bb["instructions"] = new_insts
    return json.dumps(bir).encode()


def _host_fold_weights(weight, bias, mix_weights, a_scales, w_scales):
    """Mirror the reference's fp32 weight mixture exactly; return
    (w_effT_f32 [1024,512], b_mix_f32 [512], w_mix [512,1024])."""
    w32 = np.asarray(weight, np.float32)
    b32 = np.asarray(bias, np.float32)
    mw = np.asarray(mix_weights, np.float32).reshape(3, 3, 2, 2)
    w_sc = np.asarray(w_scales, np.float32)

    coef_a = mw.sum(axis=(0, 1, 3))  # [2]
    coef_w = mw.sum(axis=2)  # [3,3,2]
    coef_b = mw.sum(axis=(2, 3))  # [3,3]

    w_mix = np.zeros((D_OUT, D_IN), np.float32)
    b_mix = np.zeros((D_OUT,), np.float32)
    for i, h in enumerate(HS):
        for j, nh in enumerate(NH):
            out_dim = NKV * (h // nh)
            w_pad = np.zeros((D_OUT, D_IN), np.float32)
            w_pad[:out_dim, :h] = w32[:out_dim, :h]
            b_pad = np.zeros((D_OUT,), np.float32)
            b_pad[:out_dim] = b32[:out_dim]
            for n, wb in enumerate(WB):
                qn, qp = -(2 ** (wb - 1)), 2 ** (wb - 1) - 1
                xs = w_pad / w_sc[n]
                xc = np.clip(xs, np.float32(qn), np.float32(qp))
                fq = np.rint(xc) * w_sc[n]
                w_mix = w_mix + coef_w[i, j, n] * fq
            b_mix = b_mix + coef_b[i, j] * b_pad

    s = np.float64(coef_a[0]) + np.float64(coef_a[1])
    w_eff = s * w_mix.astype(np.float64)  # [512, 1024]
    w_effT = np.ascontiguousarray(w_eff.T).astype(np.float32)  # [1024, 512]
    return w_effT, b_mix, w_mix


def _build_nc():
    f32, f16, f8 = mybir.dt.float32, mybir.dt.float16, mybir.dt.float8e4
    DR = mybir.MatmulPerfMode.DoubleRow
    nc = bass.Bass("TRN2", target_bir_lowering=False, debug=False)

    # xq[p, kc, tbp, tb2, i, s]: fp8 of q*2^-k with PERMUTED row
    # k' = kc*256 + i*128 + p, token = (tbp*2 + tb2)*512 + s
    xq_d = nc.dram_tensor(
        "xq", [128, KC, TB // 2, 2, 2, 512], f8, kind="ExternalInput"
    ).ap()
    # wq[p, c, i, o]: chunks 0-3 = hi rows, 4-5 = lo rows 0..511 (top
    # residual-power rows after the permutation)
    wq_d = nc.dram_tensor("wq", [128, NCH, 2, D_OUT], f8, kind="ExternalInput").ap()
    # bv[p, oc]: bias for output row oc*128 + p
    bv_d = nc.dram_tensor("bv", [128, OC], f32, kind="ExternalInput").ap()
    # transposed output
    out_d = nc.dram_tensor("outT", [D_OUT, S], f16, kind="ExternalOutput").ap()

    with tile.TileContext(nc) as tc:
        with (
            tc.tile_pool(name="const", bufs=1) as cpool,
            tc.tile_pool(name="op", bufs=8) as opool,
            tc.tile_pool(name="ps", bufs=8, space="PSUM") as pspool,
        ):
            # wq chunk 0 heads the fast-starting sync queue (the first
            # matmul blocks on it); remaining weight chunks ride gpsimd;
            # x tiles stream on sync/scalar in consumption order
            wq_sb = {}
            xq_sb = {}
            for c in range(NCH):
                wq_sb[c] = cpool.tile([128, 2, D_OUT], f8, name=f"wq{c}")
            nc.sync.dma_start(out=wq_sb[0][:], in_=wq_d[:, 0])
            for c in range(1, NCH):
                nc.gpsimd.dma_start(out=wq_sb[c][:], in_=wq_d[:, c])
            for kc in range(KC):
                for tbp in range(TB // 2):
                    t = cpool.tile([128, 2, 2, 512], f8, name=f"xq{kc}_{tbp}")
                    dma_eng = nc.sync if tbp % 2 == 0 else nc.scalar
                    dma_eng.dma_start(out=t[:], in_=xq_d[:, kc, tbp])
                    xq_sb[(kc, 2 * tbp)] = t
                    xq_sb[(kc, 2 * tbp + 1)] = t
            bv_sb = cpool.tile([128, OC], f32)
            nc.gpsimd.dma_start(out=bv_sb[:], in_=bv_d[:])

            for oc in range(OC):
                ps = [pspool.tile([128, S // TB], f32, tag="ps", name=f"ps{oc}_{i}") for i in range(TB)]
                # oc 0 consumes chunks in DMA-arrival order (the gpsimd
                # queue delivers its x tiles last); accumulation order is
                # irrelevant to the result
                c_seq = list(range(NCH))
                for ci, c in enumerate(c_seq):
                    kc = c if c < KC else c - KC
                    last = ci == NCH - 1
                    lhsT = wq_sb[c][:, :, oc * 128 : (oc + 1) * 128]
                    # consume sync-queue tiles (tbp even -> tb 0,1,4,5) first:
                    # the scalar queue's first x tiles trail its ACT-table load
                    for tb in (0, 1, 4, 5, 2, 3, 6, 7):
                        nc.tensor.matmul(
                            ps[tb][:],
                            lhsT=lhsT,
                            rhs=xq_sb[(kc, tb)][:, tb % 2],
                            start=(ci == 0),
                            stop=last,
                            perf_mode=DR,
                        )
                        if last:
                            o_sb = opool.tile([128, 512], f16, tag="o")
                            # alternate evictions DVE/ACT, stores over
                            # two queues: halves the post-matmul tail
                            if tb % 2 == 0:
                                nc.vector.tensor_scalar_add(
                                    o_sb[:],
                                    ps[tb][:],
                                    bv_sb[:, oc : oc + 1],
                                )
                            else:
                                nc.scalar.activation(
                                    o_sb[:],
                                    ps[tb][:],
                                    mybir.ActivationFunctionType.Identity,
                                    bias=bv_sb[:, oc : oc + 1],
                                    scale=1.0,
                                )
                            if oc == OC - 1:
                                # last chunk: keep the slow-draining gpsimd
                                # queue out of the critical tail
                                dma_eng = nc.sync if tb % 2 == 0 else nc.scalar
                            else:
                                dma_eng = nc.gpsimd if tb % 2 == 0 else nc.sync
                            dma_eng.dma_start(
                                out=out_d[
                                    oc * 128 : (oc + 1) * 128,
                                    tb * 512 : (tb + 1) * 512,
                                ],
                                in_=o_sb[:],
                            )

    orig = nc.to_json_bytes
    nc.to_json_bytes = lambda: _split_multi_waits(orig())
    return nc


_NC_CACHE = None


def _fq32(x, scale, bits):
    """fp32 fake_quant forward value, matching the reference bitwise."""
    qn, qp = -(2 ** (bits - 1)), 2 ** (bits - 1) - 1
    xs = (np.asarray(x, np.float32) / np.float32(scale)).astype(np.float32)
    xc = np.clip(xs, np.float32(qn), np.float32(qp))
    return (np.rint(xc) * np.float32(scale)).astype(np.float32)


def _x_mix_ref(x, mix_weights, a_scales):
    """The reference's activation mixture, in fp32."""
    mw = np.asarray(mix_weights, np.float32).reshape(3, 3, 2, 2)
    coef_a = mw.sum(axis=(0, 1, 3))
    xm = coef_a[0] * _fq32(x, a_scales[0], AB[0])
    return (xm + coef_a[1] * _fq32(x, a_scales[1], AB[1])).astype(np.float32)


def _prepare_inputs(x, weight, bias, mix_weights, a_scales, w_scales):
    """Host-side prep shared by kernel() and test.py. Returns
    (in_maps, wsum32 [1024,512], w_mix, b_mix, inv_scale)."""
    w_effT, b_mix, w_mix = _host_fold_weights(
        weight, bias, mix_weights, a_scales, w_scales
    )

    maxw = max(float(np.abs(w_effT).max()), 1e-30)
    k = int(np.clip(np.floor(np.log2(16.0 / maxw)), 0, 9))
    sc = np.float32(2.0**k)
    inv = np.float32(2.0**-k)

    wS = w_effT * sc  # [1024, 512]
    # permute k-rows so the rows with the largest fp8 residual power come
    # first; the lo correction pass covers only the top 512 of them
    # (measured end-to-end rel err 1.04e-2 vs the 2e-2 gate)
    wlo_full = wS - wS.astype(F8).astype(np.float32)
    perm = np.argsort(-((wlo_full**2).sum(axis=1)), kind="stable")
    wS_p = np.ascontiguousarray(wS[perm])
    whi = wS_p.astype(F8)
    whi32 = whi.astype(np.float32)
    wlo = (wS_p - whi32).astype(F8)
    wlo32 = wlo.astype(np.float32)
    wsum_p = whi32.copy()
    wsum_p[: 256 * LO_C] += wlo32[: 256 * LO_C]
    wsum32 = np.empty_like(wsum_p)
    wsum32[perm] = wsum_p  # original-k indexing (for the host patch)

    def to_pio(w8):  # [256*n, 512] -> [128, n, 2, 512]
        n = w8.shape[0] // 256
        return w8.reshape(n, 2, 128, D_OUT).transpose(2, 0, 1, 3)

    wq = np.ascontiguousarray(
        np.concatenate([to_pio(whi), to_pio(wlo[: 256 * LO_C])], axis=1)
    )  # [128, NCH, 2i, 512]

    bv = np.ascontiguousarray(
        b_mix.reshape(OC, 128).T.astype(np.float32)
    )  # [128, OC]

    # q * 2^-k in fp8 with permuted k, laid out [p, kc, tbp, tb2, i, s]
    q8 = (np.rint(np.asarray(x, np.float32)) * inv).astype(F8)  # [B, S, D_IN]
    in_maps = []
    for b in range(N_CORES):
        xb = q8[b][:, perm].reshape(TB // 2, 2, 512, KC, 2, 128)
        xq = np.ascontiguousarray(xb.transpose(5, 3, 0, 1, 4, 2))
        in_maps.append({"xq": xq, "wq": wq, "bv": bv})
    return in_maps, wsum32, w_mix, b_mix, inv


def kernel(x, weight, bias, mix_weights, a_scales, w_scales):
    global _NC_CACHE
    x = np.asarray(x, np.float32)
    assert x.shape == (B, S, D_IN)
    a_sc = np.asarray(a_scales, np.float32)

    in_maps, wsum32, w_mix, b_mix, inv = _prepare_inputs(
        x, weight, bias, mix_weights, a_scales, w_scales
    )

    if not np.all(a_sc == np.float32(1.0)):
        # General-scale fallback (benchmark inputs always have a_scales == 1):
        # compute the reference mixture on host in fp32.
        x_mix = _x_mix_ref(x, mix_weights, a_scales)
        return (np.einsum("bsi,oi->bso", x_mix, w_mix) + b_mix).astype(np.float32)

    if _NC_CACHE is None:
        _NC_CACHE = _build_nc()
    nc = _NC_CACHE

    try:
        res = run_bass_kernel_spmd(nc, in_maps, list(range(N_CORES)))
    except Exception:
        # one retry for transient device errors
        res = run_bass_kernel_spmd(nc, in_maps, list(range(N_CORES)))
    out = np.stack(
        [
            np.asarray(res.results[b]["outT"]).astype(np.float32).T
            for b in range(N_CORES)
        ],
        axis=0,
    )

    # Exact host patch for |x| >= 7.49, where rint(x) differs from the
    # reference's clipped fake-quants (x ~ N(0,1) in the benchmark: never
    # triggers; keeps kernel() correct for arbitrary inputs).
    idx = np.argwhere(np.abs(x) >= 7.49)
    if len(idx):
        for b, t, i in idx:
            xv = x[b, t, i]
            ref_xmix = _x_mix_ref(xv, mix_weights, a_sc)
            # what the device computed for this element (same IEEE ops)
            dev_q = np.float32(F8.type(np.float32(np.rint(xv)) * inv))
            out[b, t, :] += ref_xmix * w_mix[:, i] - dev_q * wsum32[i, :]
    return out
